# revision 84
# baseline (speedup 1.0000x reference)
"""BERT per-word mean-pool (segment reduce) on 8 Trainium2 NeuronCores.

Problem: output[B=64, S=512, E=768] f32, mappings[B, W=255] int32 (values 1 or 2).
Per sentence, strip [CLS]/[SEP], mean-pool contiguous BPE spans into word vectors.

Key identity: every word's span has 1 or 2 BPE rows.  With s = span start,
    out[w] = (1/m) * (hs rows s .. s+m-1 summed),  m in {1, 2}.

Sharding: pure data parallel, 8 sentences per core, no cross-core comms.
All device data in bf16 (tolerance 2e-2 >> bf16's ~2e-3); host casts.

Two competitive kernels, HW-measured:

* mode "mm" (default): segment-sum as matmul. NO gathers, NO Q7 ucode --
  dodges the ~16us ucode-ready gate that floors every dma_gather kernel.
  Contiguous loads of x (SWDGE+HWDGE alternating); the 0/1 segment matrix
  M^T[t, w] = [s_w <= t] - [s_{w+1} <= t] is built on-chip from an uploaded
  boundary row (2 DVE ops per 128-row k-tile); TensorE accumulates
  out = M^T.T @ x per (sentence, 128-word tile) in PSUM; ACT evicts with a
  per-partition 1/m scale; HWDGE stores.

* mode "wh": windowed ucode gather. One 2-row-window descriptor per word
  (InstDMAGatherAnt), x pre-scaled by 1/m on host, single fused DVE
  scalar_tensor_tensor per word-column: out = t1*[m==2] + t0.

Raw indirect InstDMACopy (modes ind/acc/wind) is broken on this runtime:
the DGE consumes ONE offset per partition of the out AP and fetches
consecutive rows after it (HW-probed); a 3D out AP makes it worse. Do not
use those modes.
"""

import numpy as np

from concourse import bacc, bass, mybir, tile
from concourse.bass_utils import run_bass_kernel_spmd

B, S, W, E = 64, 512, 255, 768
NCORES = 8
BPC = B // NCORES            # sentences per core
NW = BPC * W                 # 2040 real words per core
NWP = 2048                   # padded word count (multiple of 512)
NCHUNK = 4                   # chunks per core
CPW = NWP // NCHUNK          # 512 words per chunk
JJ = CPW // 128              # 4 words per partition per chunk
ROWS = BPC * S               # 4096 input rows per core
NIDX = 2 * CPW               # 1024 gather indices per chunk (A then B)

_F32 = mybir.dt.float32
_BF16 = mybir.dt.bfloat16
_I16 = mybir.dt.int16

_NPBF16 = None


def _np_bf16():
    global _NPBF16
    if _NPBF16 is None:
        import ml_dtypes

        _NPBF16 = ml_dtypes.bfloat16
    return _NPBF16


def _build_nc(reps=1, bufs=2, order="pc", nq=1, mode="ab", nchunk=NCHUNK,
              merged_idx=False, warm=False, hscale=False, sizes=None,
              dt="f32", split0=False):
    _DT = _F32 if dt == "f32" else _BF16
    if sizes is not None:
        return _build_nc_sized(reps, bufs, sizes, hscale)
    if mode == "ind":
        return _build_nc_ind(reps, bufs, dt, nchunk, hscale)
    if mode == "acc":
        return _build_nc_acc(reps, bufs, dt, nchunk)
    if mode == "wind":
        return _build_nc_wind(reps, bufs, dt, nchunk)
    if mode == "mm":
        return _build_nc_mm(reps)
    nc = bacc.Bacc(
        "TRN2",
        target_bir_lowering=False,
        debug=False,
        num_devices=NCORES,
        num_swdge_queues=nq,
    )
    x = nc.dram_tensor("x", [ROWS, E], _DT, kind="ExternalInput").ap()
    # indices are int16, wrapped [i%16, i//16] into 16 partitions and
    # replicated 8x down to 128 partitions (Q7 core replication).
    cpw = NWP // nchunk
    jj = cpw // 128
    nidx = 2 * cpw if mode == "ab" else cpw
    idx = nc.dram_tensor(
        "idx", [nchunk, 128, nidx // 16], _I16, kind="ExternalInput"
    ).ap()
    _RDT = _F32 if mode == "ws" else _DT  # tensor_scalar wants f32 scalars
    if mode in ("win", "ws"):
        # per word w: rw[p, 2c] = 1/m(w), rw[p, 2c+1] = (m(w)-1)/m(w)
        rw = nc.dram_tensor(
            "rw", [nchunk, 128, 2 * jj], _RDT, kind="ExternalInput"
        ).ap()
    if mode == "wh":
        # per word w: bt[p, c] = [m(w) == 2]; x rows pre-scaled by 1/m on host
        bt = nc.dram_tensor(
            "bt", [nchunk, 128, jj], _DT, kind="ExternalInput"
        ).ap()
    y = nc.dram_tensor("y", [NWP, E], _DT, kind="ExternalOutput").ap()

    with tile.TileContext(nc) as tc:
        with (
            tc.tile_pool(name="idxp", bufs=1) as ipool,
            tc.tile_pool(name="io", bufs=bufs) as pool,
        ):
            if warm:
                # dummy 16-index gather issued first: triggers the Q7
                # ucode IRAM fetch (~6us) while the idx loads stream in,
                # so the first real gather isn't stalled on it.
                wi = ipool.tile([128, 1], _I16, tag="warmi")
                nc.gpsimd.memset(wi[:], 0)
                wo = ipool.tile([128, E], _DT, tag="warmo")
                nc.gpsimd.dma_gather(
                    wo[:].rearrange("p (c e) -> p c e", e=E),
                    x[:, :], wi[:], 16, 16, E,
                )
            its, rts = [], []
            ncols = nidx // 16
            if merged_idx:
                its = []
                if split0:
                    # chunk-0 idx as its own tiny first DMA so the first
                    # gather isn't gated on the full idx upload
                    it0 = ipool.tile([128, ncols], _I16, tag="it0")
                    nc.sync.dma_start(out=it0[:], in_=idx[0])
                    its.append(it0[:])
                    itall = ipool.tile(
                        [128, (nchunk - 1) * ncols], _I16, tag="itall"
                    )
                    nc.sync.dma_start(
                        out=itall[:].rearrange(
                            "p (q s) -> p q s", q=nchunk - 1
                        ),
                        in_=idx[1:].rearrange("q p s -> p q s"),
                    )
                    its += [
                        itall[:, q * ncols : (q + 1) * ncols]
                        for q in range(nchunk - 1)
                    ]
                else:
                    itall = ipool.tile([128, nchunk * ncols], _I16, tag="itall")
                    nc.sync.dma_start(
                        out=itall[:].rearrange("p (q s) -> p q s", q=nchunk),
                        in_=idx.rearrange("q p s -> p q s"),
                    )
                    its = [
                        itall[:, q * ncols : (q + 1) * ncols]
                        for q in range(nchunk)
                    ]
            else:
                for q in range(nchunk):
                    it = ipool.tile([128, ncols], _I16, tag=f"it{q}")
                    nc.sync.dma_start(out=it[:], in_=idx[q])
                    its.append(it[:])
            if mode in ("win", "ws"):
                # single merged weight load (one HWDGE DMA for all chunks)
                rtall = ipool.tile([128, nchunk * 2 * jj], _RDT, tag="rtall")
                nc.sync.dma_start(
                    out=rtall[:].rearrange("p (q s) -> p q s", q=nchunk),
                    in_=rw.rearrange("q p s -> p q s"),
                )
                rts = [
                    rtall[:, q * 2 * jj : (q + 1) * 2 * jj]
                    for q in range(nchunk)
                ]
            if mode == "wh":
                btall = ipool.tile([128, nchunk * jj], _DT, tag="btall")
                nc.sync.dma_start(
                    out=btall[:].rearrange("p (q s) -> p q s", q=nchunk),
                    in_=bt.rearrange("q p s -> p q s"),
                )
            for _rep in range(reps):
                for q in range(nchunk):
                    if mode == "ab":
                        # gathered slot i -> T[i % 128, i // 128, :]
                        # i = c*128 + p:  c in 0..3 -> first-BPE row of word
                        # w = q*512 + p*4 + c;  c in 4..7 -> last-BPE row.
                        t = pool.tile([128, 2 * jj * E], _DT, tag="t")
                        nc.gpsimd.dma_gather(
                            t[:].rearrange("p (c e) -> p c e", e=E),
                            x[:, :],
                            its[q],
                            nidx,
                            nidx,
                            E,
                            queue_num=q % nq,
                        )
                        c = pool.tile([128, jj * E], _DT, tag="c")
                        nc.vector.tensor_add(
                            out=c[:], in0=t[:, : jj * E], in1=t[:, jj * E :]
                        )
                        if not hscale:
                            nc.scalar.mul(c[:], c[:], 0.5)
                    else:
                        # one 2-row window [s, s+2) per word, 6KB descriptors;
                        # out[w] = win[0]*r1 + win[1]*r2 kills the junk row
                        # (m=1: r=(1,0); m=2: r=(.5,.5)).
                        t = pool.tile([128, 2 * jj * E], _DT, tag="t")
                        xw = bass.AP(x.tensor, 0, [[E, ROWS - 1], [1, 2 * E]])
                        nc.gpsimd.dma_gather(
                            t[:].rearrange("p (c e) -> p c e", e=2 * E),
                            xw,
                            its[q],
                            cpw,
                            cpw,
                            2 * E,
                            elem_step=E,
                            queue_num=q % nq,
                        )
                        c = pool.tile([128, jj * E], _DT, tag="c")
                        if mode == "wh":
                            # x rows pre-scaled by 1/m on host; one fused DVE
                            # op per word-column: out = t1*[m==2] + t0, all
                            # APs contiguous [128, E] (full-rate DVE).
                            for j in range(jj):
                                nc.vector.scalar_tensor_tensor(
                                    out=c[:, j * E : (j + 1) * E],
                                    in0=t[:, (2 * j + 1) * E : (2 * j + 2) * E],
                                    scalar=btall[:, q * jj + j : q * jj + j + 1],
                                    in1=t[:, 2 * j * E : (2 * j + 1) * E],
                                    op0=mybir.AluOpType.mult,
                                    op1=mybir.AluOpType.add,
                                )
                        elif mode == "ws":
                            # device weights, contiguous slices: per word-col
                            # c3 = t1*r2; c = t0*r1 + c3
                            c3 = pool.tile([128, jj * E], _DT, tag="c3")
                            for j in range(jj):
                                nc.vector.tensor_scalar_mul(
                                    c3[:, j * E : (j + 1) * E],
                                    t[:, (2 * j + 1) * E : (2 * j + 2) * E],
                                    rts[q][:, 2 * j + 1 : 2 * j + 2],
                                )
                                nc.vector.scalar_tensor_tensor(
                                    out=c[:, j * E : (j + 1) * E],
                                    in0=t[:, 2 * j * E : (2 * j + 1) * E],
                                    scalar=rts[q][:, 2 * j : 2 * j + 1],
                                    in1=c3[:, j * E : (j + 1) * E],
                                    op0=mybir.AluOpType.mult,
                                    op1=mybir.AluOpType.add,
                                )
                        else:
                            t3 = t[:].rearrange("p (c e) -> p c e", e=2 * E)
                            r3 = rts[q].rearrange("p (c f) -> p c f", f=2)
                            c3 = pool.tile([128, jj * E], _DT, tag="c3")
                            cv = c[:].rearrange("p (j e) -> p j e", e=E)
                            c3v = c3[:].rearrange("p (j e) -> p j e", e=E)
                            nc.vector.tensor_tensor(
                                out=cv,
                                in0=t3[:, :, :E],
                                in1=r3[:, :, 0:1].to_broadcast([128, jj, E]),
                                op=mybir.AluOpType.mult,
                            )
                            nc.vector.tensor_tensor(
                                out=c3v,
                                in0=t3[:, :, E:],
                                in1=r3[:, :, 1:2].to_broadcast([128, jj, E]),
                                op=mybir.AluOpType.mult,
                            )
                            nc.vector.tensor_add(out=c[:], in0=c[:], in1=c3[:])
                    ychunk = y[q * cpw : (q + 1) * cpw, :]
                    if order == "pc":
                        nc.sync.dma_start(
                            out=ychunk.rearrange("(p j) e -> p (j e)", p=128),
                            in_=c[:],
                        )
                    else:
                        nc.sync.dma_start(
                            out=ychunk.rearrange("(j p) e -> p j e", p=128),
                            in_=c[:].rearrange("p (j e) -> p j e", e=E),
                        )
    nc.compile()
    return nc


def _build_nc_ind(reps, bufs, dt="f32", nchunk=4, hscale=False, tbufs=1):
    """Native SWDGE indirect gather (no ucode library -> no ~16us Q7 ucode
    ready gate). One InstDMACopy per chunk reading a column slice of a single
    int32 offset tile; slices advance monotonically (the HW-verified-safe
    pattern). Per chunk: first wpp cols = first-BPE rows, next wpp cols =
    last-BPE rows of words w = q*cpw + p*wpp + j."""
    _DT = _F32 if dt == "f32" else _BF16
    cpw = NWP // nchunk
    wpp = cpw // 128              # words per partition per chunk
    nc = bacc.Bacc(
        "TRN2", target_bir_lowering=False, debug=False, num_devices=NCORES
    )
    x = nc.dram_tensor("x", [ROWS, E], _DT, kind="ExternalInput").ap()
    idx = nc.dram_tensor("idx", [128, 2 * wpp * nchunk], mybir.dt.int32,
                         kind="ExternalInput").ap()
    y = nc.dram_tensor("y", [NWP, E], _DT, kind="ExternalOutput").ap()
    with tile.TileContext(nc) as tc:
        with (
            tc.tile_pool(name="idxp", bufs=1) as ipool,
            tc.tile_pool(name="tp", bufs=tbufs) as tpool,
            tc.tile_pool(name="io", bufs=bufs) as pool,
        ):
            itall = ipool.tile([128, 2 * wpp * nchunk], mybir.dt.int32,
                               tag="itall")
            nc.sync.dma_start(out=itall[:], in_=idx[:, :])
            for _rep in range(reps):
                for q in range(nchunk):
                    # tbufs=1 pool: WAR dep guarantees at most one indirect
                    # DMA in flight (two concurrent ones corrupt offsets).
                    t = tpool.tile([128, 2 * wpp * E], _DT, tag="t")
                    nc.gpsimd.indirect_dma_start(
                        # 3D out AP: one offset consumed per (p, c) row slot.
                        # A flat [128, 2*wpp*E] out makes the DGE take ONE
                        # offset per partition and fetch consecutive rows
                        # (HW-probed failure mode).
                        out=t[:].rearrange("p (c e) -> p c e", e=E),
                        out_offset=None,
                        in_=x[:, :],
                        in_offset=bass.IndirectOffsetOnAxis(
                            ap=itall[:, 2 * wpp * q : 2 * wpp * (q + 1)],
                            axis=0,
                        ),
                    )
                    c = pool.tile([128, wpp * E], _DT, tag="c")
                    nc.vector.tensor_add(
                        out=c[:], in0=t[:, : wpp * E], in1=t[:, wpp * E :]
                    )
                    if not hscale:
                        nc.scalar.mul(c[:], c[:], 0.5)
                    nc.sync.dma_start(
                        out=y[q * cpw : (q + 1) * cpw, :].rearrange(
                            "(p j) e -> p (j e)", p=128
                        ),
                        in_=c[:],
                    )
    nc.compile()
    return nc


def _build_nc_wind(reps, bufs, dt, nchunk, tbufs=1):
    """Native indirect gather of 2-row windows (one InstDMACopy per chunk,
    offsets in raw elements via a 1-D source view; coef=1), then the wh-style
    fused STT combine (x host-scaled by 1/m, beta kills junk rows)."""
    _DT = _F32 if dt == "f32" else _BF16
    cpw = NWP // nchunk
    wpp = cpw // 128
    nc = bacc.Bacc(
        "TRN2", target_bir_lowering=False, debug=False, num_devices=NCORES
    )
    x = nc.dram_tensor("x", [ROWS, E], _DT, kind="ExternalInput").ap()
    idx = nc.dram_tensor("idx", [128, wpp * nchunk], mybir.dt.int32,
                         kind="ExternalInput").ap()
    bt = nc.dram_tensor("bt", [nchunk, 128, wpp], _DT,
                        kind="ExternalInput").ap()
    y = nc.dram_tensor("y", [NWP, E], _DT, kind="ExternalOutput").ap()
    with tile.TileContext(nc) as tc:
        with (
            tc.tile_pool(name="idxp", bufs=1) as ipool,
            tc.tile_pool(name="tp", bufs=tbufs) as tpool,
            tc.tile_pool(name="io", bufs=bufs) as pool,
        ):
            itall = ipool.tile([128, wpp * nchunk], mybir.dt.int32,
                               tag="itall")
            nc.sync.dma_start(out=itall[:], in_=idx[:, :])
            btall = ipool.tile([128, nchunk * wpp], _DT, tag="btall")
            nc.sync.dma_start(
                out=btall[:].rearrange("p (q s) -> p q s", q=nchunk),
                in_=bt.rearrange("q p s -> p q s"),
            )
            for _rep in range(reps):
                for q in range(nchunk):
                    t = tpool.tile([128, 2 * wpp * E], _DT, tag="t")
                    # coef = E (row stride); each (p, c) out slot pulls 2E
                    # contiguous elements = the 2-row window [s, s+2)
                    nc.gpsimd.indirect_dma_start(
                        out=t[:].rearrange("p (c e) -> p c e", e=2 * E),
                        out_offset=None,
                        in_=x[:, :],
                        in_offset=bass.IndirectOffsetOnAxis(
                            ap=itall[:, wpp * q : wpp * (q + 1)], axis=0),
                    )
                    c = pool.tile([128, wpp * E], _DT, tag="c")
                    for j in range(wpp):
                        nc.vector.scalar_tensor_tensor(
                            out=c[:, j * E : (j + 1) * E],
                            in0=t[:, (2 * j + 1) * E : (2 * j + 2) * E],
                            scalar=btall[:, q * wpp + j : q * wpp + j + 1],
                            in1=t[:, 2 * j * E : (2 * j + 1) * E],
                            op0=mybir.AluOpType.mult,
                            op1=mybir.AluOpType.add,
                        )
                    nc.sync.dma_start(
                        out=y[q * cpw : (q + 1) * cpw, :].rearrange(
                            "(p j) e -> p (j e)", p=128),
                        in_=c[:],
                    )
    nc.compile()
    return nc


def _build_nc_mm(reps=1, xbufs=1, pbufs=4, obufs=6):
    """Segment-sum as matmul: NO gathers, NO Q7 ucode (dodges the ~16us
    ucode-ready gate). Contiguous HWDGE loads of x; the 0/1 segment matrix
    M^T[t, w] = [s_w <= t] - [s_{w+1} <= t] is built on-chip from an
    uploaded (replicated) boundary row via two DVE ops per k-tile; TensorE
    computes out = M^T.T @ x per (sentence, word-tile); PSUM is evicted with
    a per-partition 1/m scale (the span mean) straight to bf16 and stored.

    Word layout: y row = sent*256 + w (pad word 255 per sentence interleaved).
    """
    NS = BPC                      # sentences per core = 8
    KT = S // 128                 # k-tiles per sentence = 4
    WT = 2                        # word-tiles per sentence (256 words)
    SW = 257                      # boundary cols per sentence (s_0..s_255, sentinel)
    nc = bacc.Bacc(
        "TRN2", target_bir_lowering=False, debug=False, num_devices=NCORES
    )
    x = nc.dram_tensor("x", [ROWS, E], _BF16, kind="ExternalInput").ap()
    sth = nc.dram_tensor("sth", [1, NS * SW], mybir.dt.float16,
                         kind="ExternalInput").ap()
    gcol = nc.dram_tensor("gcol", [128, KT], _F32, kind="ExternalInput").ap()
    msc = nc.dram_tensor("msc", [128, NS * WT], _F32,
                         kind="ExternalInput").ap()
    y = nc.dram_tensor("y", [NS * 256, E], _BF16, kind="ExternalOutput").ap()

    with tile.TileContext(nc) as tc:
        with (
            tc.tile_pool(name="const", bufs=1) as cpool,
            tc.tile_pool(name="xs", bufs=xbufs) as xpool,
            tc.tile_pool(name="m", bufs=1) as mpool,
            tc.tile_pool(name="ps", bufs=pbufs, space="PSUM") as ppool,
            tc.tile_pool(name="out", bufs=obufs) as opool,
        ):
            # x loads split across the SWDGE ring (gpsimd) and the ACT
            # engine's own HWDGE ring (qActDynamicHW) -- two descriptor
            # paths inject in parallel; the sync ring stays FIFO-clean for
            # the stores (mixing big loads with stores on one ring
            # head-of-line-blocks them; HW-measured +14us)
            # boundary row: 4KB flat upload + PE broadcast to 128 partitions
            # (a [128, 2056] replicated upload costs 0.53MB of HBM stream;
            # ones-matmul replication costs ~0)
            sfl = cpool.tile([1, NS * SW], mybir.dt.float16, tag="sfl")
            nc.sync.dma_start(out=sfl[:], in_=sth[:, :])
            ones = cpool.tile([1, 128], mybir.dt.float16, tag="ones")
            nc.vector.memset(ones[:], 1.0)
            stht = cpool.tile([128, NS * SW], mybir.dt.float16, tag="sth")
            off = 0
            while off < NS * SW:
                n = min(384, NS * SW - off)
                pb = ppool.tile([128, 384], _F32, tag="p0")
                nc.tensor.matmul(
                    pb[:, :n], ones[:], sfl[:, off : off + n],
                    start=True, stop=True,
                )
                nc.vector.tensor_copy(out=stht[:, off : off + n], in_=pb[:, :n])
                off += n
            gct = cpool.tile([128, KT], _F32, tag="gcol")
            nc.sync.dma_start(out=gct[:], in_=gcol[:, :])
            msct = cpool.tile([128, NS * WT], _F32, tag="msc")
            nc.sync.dma_start(out=msct[:], in_=msc[:, :])
            # last sentence per ring (s6 scalar, s7 gpsimd) split (k0-2)+(k3):
            # its wt0 matmuls overlap the final k3 transfer, shortening the
            # post-last-load chain
            xts = []
            xbs = {}
            for s in range(NS):
                eng = nc.scalar if s % 2 == 0 else nc.gpsimd
                xin = x[s * S : (s + 1) * S, :].rearrange(
                    "(k p) e -> p k e", p=128
                )
                if s >= NS - 2:
                    xa = xpool.tile([128, 3 * E], _BF16, tag=f"xa{s}")
                    eng.dma_start(
                        out=xa[:].rearrange("p (k e) -> p k e", e=E),
                        in_=x[s * S : s * S + 384, :].rearrange(
                            "(k p) e -> p k e", p=128
                        ),
                    )
                    xb = xpool.tile([128, E], _BF16, tag=f"xb{s}")
                    eng.dma_start(out=xb[:], in_=x[s * S + 384 : (s + 1) * S, :])
                    xts.append(xa)
                    xbs[s] = xb
                else:
                    xt = xpool.tile([128, KT * E], _BF16, tag=f"x{s}")
                    eng.dma_start(
                        out=xt[:].rearrange("p (k e) -> p k e", e=E),
                        in_=xin,
                    )
                    xts.append(xt)
            for _rep in range(reps):
                # cmp_k[p, sent*SW + w] = [s_w <= p + 128k]  (0/1 bf16)
                cmps = []
                for k in range(KT):
                    ck = mpool.tile([128, NS * SW], _BF16, tag=f"cmp{k}")
                    nc.vector.tensor_scalar(
                        ck[:], stht[:], gct[:, k : k + 1], None,
                        op0=mybir.AluOpType.is_le,
                    )
                    cmps.append(ck)
                # M^T block (k, sent): [128 rows, 256 words]
                mts = {}
                for k in range(KT):
                    for s in range(NS):
                        mt = mpool.tile([128, 256], _BF16, tag=f"m{k}_{s}")
                        nc.vector.tensor_tensor(
                            out=mt[:],
                            in0=cmps[k][:, s * SW : s * SW + 256],
                            in1=cmps[k][:, s * SW + 1 : s * SW + 257],
                            op=mybir.AluOpType.subtract,
                        )
                        mts[(k, s)] = mt
                # per (sent, wt): 3 k-blocks x 2 N-halves accumulate in PSUM
                for s in range(NS):
                    for wt in range(WT):
                        ks = (0, 1, 2) if wt == 0 else (1, 2, 3)
                        pts = []
                        for h in range(2):
                            pt = ppool.tile([128, 384], _F32, tag=f"p{h}")
                            for i, k in enumerate(ks):
                                if k == 3 and s in xbs:
                                    rhs = xbs[s][:, h * 384 : (h + 1) * 384]
                                else:
                                    rhs = xts[s][:, k * E + h * 384 :
                                                 k * E + (h + 1) * 384]
                                nc.tensor.matmul(
                                    pt[:],
                                    mts[(k, s)][:, wt * 128 : (wt + 1) * 128],
                                    rhs,
                                    start=(i == 0),
                                    stop=(i == len(ks) - 1),
                                )
                            pts.append(pt)
                        ot = opool.tile([128, E], _BF16, tag="o")
                        col = s * WT + wt
                        # PSUM eviction + 1/m scale split across DVE and ACT
                        # (a single engine's 32-op eviction chain paces the
                        # whole tail at ~20us; HW-traced)
                        nc.vector.tensor_scalar_mul(
                            ot[:, 0:384], pts[0][:], msct[:, col : col + 1]
                        )
                        nc.scalar.mul(
                            ot[:, 384:768], pts[1][:], msct[:, col : col + 1]
                        )
                        nc.sync.dma_start(
                            out=y[s * 256 + wt * 128 : s * 256 + (wt + 1) * 128, :],
                            in_=ot[:],
                        )
    nc.compile()
    return nc


def _build_nc_acc(reps, bufs, dt="f32", nchunk=4):
    """Native indirect gathers, zero compute engines: host uploads x/2, the
    B gather CCE-accumulates onto the A gather in SBUF, and the only
    consumer is a DMA store (the HW-verified-safe consumer)."""
    _DT = _F32 if dt == "f32" else _BF16
    cpw = NWP // nchunk
    wpp = cpw // 128
    nc = bacc.Bacc(
        "TRN2", target_bir_lowering=False, debug=False, num_devices=NCORES
    )
    x = nc.dram_tensor("x", [ROWS, E], _DT, kind="ExternalInput").ap()
    idx = nc.dram_tensor("idx", [128, 2 * wpp * nchunk], mybir.dt.int32,
                         kind="ExternalInput").ap()
    y = nc.dram_tensor("y", [NWP, E], _DT, kind="ExternalOutput").ap()
    with tile.TileContext(nc) as tc:
        with (
            tc.tile_pool(name="idxp", bufs=1) as ipool,
            tc.tile_pool(name="io", bufs=bufs) as pool,
        ):
            itall = ipool.tile([128, 2 * wpp * nchunk], mybir.dt.int32,
                               tag="itall")
            nc.sync.dma_start(out=itall[:], in_=idx[:, :])
            for _rep in range(reps):
                for q in range(nchunk):
                    o = 2 * wpp * q
                    t = pool.tile([128, wpp * E], _DT, tag="t")
                    t3 = t[:].rearrange("p (c e) -> p c e", e=E)
                    nc.gpsimd.indirect_dma_start(
                        out=t3, out_offset=None, in_=x[:, :],
                        in_offset=bass.IndirectOffsetOnAxis(
                            ap=itall[:, o : o + wpp], axis=0),
                    )
                    nc.gpsimd.indirect_dma_start(
                        out=t3, out_offset=None, in_=x[:, :],
                        in_offset=bass.IndirectOffsetOnAxis(
                            ap=itall[:, o + wpp : o + 2 * wpp], axis=0),
                        compute_op=mybir.AluOpType.add,
                    )
                    nc.sync.dma_start(
                        out=y[q * cpw : (q + 1) * cpw, :].rearrange(
                            "(p j) e -> p (j e)", p=128),
                        in_=t[:],
                    )
    nc.compile()
    return nc


def _build_nc_sized(reps, bufs, sizes, hscale):
    """ab-mode kernel with per-chunk word counts `sizes` (multiples of 128
    summing to NWP). Small leading chunks start transfers sooner; small
    trailing chunks shorten the exposed compute+store tail."""
    assert sum(sizes) == NWP and all(s % 128 == 0 for s in sizes)
    nc = bacc.Bacc(
        "TRN2", target_bir_lowering=False, debug=False, num_devices=NCORES
    )
    x = nc.dram_tensor("x", [ROWS, E], _F32, kind="ExternalInput").ap()
    total_cols = sum(2 * s // 16 for s in sizes)
    idx = nc.dram_tensor("idx", [128, total_cols], _I16, kind="ExternalInput").ap()
    y = nc.dram_tensor("y", [NWP, E], _F32, kind="ExternalOutput").ap()
    jmax = max(sizes) // 128

    with tile.TileContext(nc) as tc:
        with (
            tc.tile_pool(name="idxp", bufs=1) as ipool,
            tc.tile_pool(name="io", bufs=bufs) as pool,
        ):
            itall = ipool.tile([128, total_cols], _I16, tag="itall")
            nc.sync.dma_start(out=itall[:], in_=idx[:, :])
            for _rep in range(reps):
                off_c = 0
                off_w = 0
                for s in sizes:
                    jj = s // 128
                    cols = 2 * s // 16
                    t = pool.tile([128, 2 * jmax * E], _F32, tag="t")
                    nc.gpsimd.dma_gather(
                        t[:, : 2 * jj * E].rearrange("p (c e) -> p c e", e=E),
                        x[:, :],
                        itall[:, off_c : off_c + cols],
                        2 * s,
                        2 * s,
                        E,
                    )
                    c = pool.tile([128, jmax * E], _F32, tag="c")
                    nc.vector.tensor_add(
                        out=c[:, : jj * E],
                        in0=t[:, : jj * E],
                        in1=t[:, jj * E : 2 * jj * E],
                    )
                    if not hscale:
                        nc.scalar.mul(c[:, : jj * E], c[:, : jj * E], 0.5)
                    nc.sync.dma_start(
                        out=y[off_w : off_w + s, :].rearrange(
                            "(p j) e -> p (j e)", p=128
                        ),
                        in_=c[:, : jj * E],
                    )
                    off_c += cols
                    off_w += s
    nc.compile()
    return nc


_NC = {}


def _get_nc(reps=1, bufs=2, order="pc", nq=1, mode="ab", nchunk=NCHUNK,
            merged_idx=False, warm=False, hscale=False, sizes=None,
            dt="f32", split0=False):
    key = (reps, bufs, order, nq, mode, nchunk, merged_idx, warm, hscale,
           tuple(sizes) if sizes else None, dt, split0)
    if key not in _NC:
        _NC[key] = _build_nc(reps, bufs, order, nq, mode, nchunk, merged_idx,
                             warm, hscale, sizes, dt, split0)
    return _NC[key]


def _wrap16(flat):
    """int16 index list -> [128, n/16] wrapped (i -> [i%16, i//16]) + 8x rep."""
    return np.tile(flat.reshape(-1, 16).T, (8, 1)).astype(np.int16)


def _make_in_maps(output, mappings, order="pc", mode="ab", nchunk=NCHUNK,
                  hscale=False, sizes=None, dt="f32"):
    output = np.asarray(output)
    if hscale:
        # fold the *0.5 of the span mean into the shard upload: a/2 + b/2
        # rounds identically to (a+b)/2 in f32 (halving is exact).
        output = output * np.float32(0.5)
    npdt = np.float32 if dt == "f32" else _np_bf16()
    mappings = np.asarray(mappings, dtype=np.int32)
    ends = np.cumsum(mappings, axis=1, dtype=np.int32)  # [B, W] exclusive ends
    src_a = ends - mappings + 1                         # +1: skip [CLS]
    src_b = ends                                        # (e-1) + 1
    if mode in ("wh", "wind"):
        # scale every BPE row by 1/m of its owning word (0.5/1 exact in f32,
        # single bf16 rounding after); junk rows keep scale 1 and are killed
        # on-device by beta=0.
        g = np.ones((B, S), np.float32)
        two = mappings == 2
        np.put_along_axis(
            g, src_a, np.where(two, np.float32(0.5), np.float32(1.0)), axis=1
        )
        bi, wi_ = np.nonzero(two)
        g[bi, src_a[bi, wi_] + 1] = 0.5
        output = output * g[:, :, None]
    output = np.ascontiguousarray(output.astype(npdt))

    if mode == "mm":
        in_maps = []
        gcol = (np.arange(128, dtype=np.float32)[:, None]
                + 128.0 * np.arange(4, dtype=np.float32)[None, :])
        gcol = np.ascontiguousarray(gcol)
        minv = 1.0 / mappings.astype(np.float32)          # [B, W]
        for k in range(NCORES):
            bs = slice(k * BPC, (k + 1) * BPC)
            sa = src_a[bs]                                # [8, 255] s-coords
            sth = np.empty((BPC, 257), np.float16)
            sth[:, :255] = sa
            sth[:, 255] = ends[bs, -1] + 1                # pad word start
            sth[:, 256] = 600.0                           # sentinel > 511
            sth = np.ascontiguousarray(sth.reshape(1, -1))  # [1, 2056]
            msc = np.ones((128, BPC * 2), np.float32)
            mi = minv[bs]                                 # [8, 255]
            for s_ in range(BPC):
                msc[:, s_ * 2] = mi[s_, 0:128]
                msc[:128 - 1, s_ * 2 + 1] = mi[s_, 128:255]
            xk = np.ascontiguousarray(
                output[bs].reshape(ROWS, E).astype(_np_bf16()))
            in_maps.append({"x": xk, "sth": sth, "gcol": gcol,
                            "msc": np.ascontiguousarray(msc)})
        return in_maps

    in_maps = []
    for k in range(NCORES):
        bs = slice(k * BPC, (k + 1) * BPC)
        base = (np.arange(BPC, dtype=np.int32) * S)[:, None]
        a = (src_a[bs] + base).reshape(-1)
        b = (src_b[bs] + base).reshape(-1)
        pad = np.zeros(NWP - NW, np.int32)
        a = np.concatenate([a, pad])  # [NWP] word-ordered flat row ids
        b = np.concatenate([b, pad])
        x = np.ascontiguousarray(output[bs].reshape(ROWS, E))
        if mode == "wind":
            wpp = NWP // nchunk // 128
            mm = np.concatenate(
                [mappings[bs].reshape(-1), np.ones(NWP - NW, np.int32)]
            )
            beta = (mm == 2).astype(npdt)
            # [p, q*wpp+j] = element offset of word q*cpw + p*wpp + j
            aw = a.reshape(nchunk, 128, wpp).transpose(1, 0, 2)  # [p, q, j]
            idx = np.ascontiguousarray(aw.reshape(128, -1).astype(np.int32))
            bt = np.empty((nchunk, 128, wpp), npdt)
            cpw_ = NWP // nchunk
            for q in range(nchunk):
                sl = slice(q * cpw_, (q + 1) * cpw_)
                bt[q] = beta[sl].reshape(128, wpp)
            in_maps.append({"x": x, "idx": idx, "bt": bt})
            continue
        if mode in ("ind", "acc"):
            nck = nchunk
            wpp = NWP // nck // 128
            ia = a.reshape(nck, 128, wpp).transpose(1, 0, 2)  # [p, q, j]
            ib = b.reshape(nck, 128, wpp).transpose(1, 0, 2)
            idx = np.concatenate(
                [np.concatenate([ia[:, q], ib[:, q]], axis=1)
                 for q in range(nck)],
                axis=1,
            ).astype(np.int32)  # [128, 2*wpp*nck], cols 2*wpp*q + j
            in_maps.append({"x": x, "idx": np.ascontiguousarray(idx)})
            continue
        if sizes is not None:
            segs = []
            off = 0
            for s in sizes:
                jj = s // 128
                aq = a[off : off + s].reshape(128, jj).T.ravel()
                bq = b[off : off + s].reshape(128, jj).T.ravel()
                segs.append(_wrap16(np.concatenate([aq, bq])))
                off += s
            in_maps.append({"x": x, "idx": np.concatenate(segs, axis=1)})
            continue
        cpw = NWP // nchunk
        jj = cpw // 128
        if mode == "ab":
            idx = np.empty((nchunk, 128, 2 * cpw // 16), np.int16)
            for q in range(nchunk):
                aq = a[q * cpw : (q + 1) * cpw]
                bq = b[q * cpw : (q + 1) * cpw]
                if order == "pc":
                    # gathered i = c*128 + p holds word q*cpw + p*jj + c
                    aq = aq.reshape(128, jj).T.ravel()
                    bq = bq.reshape(128, jj).T.ravel()
                # 'seq': gathered i holds word q*cpw + i (ascending rows)
                idx[q] = _wrap16(np.concatenate([aq, bq]))
            in_maps.append({"x": x, "idx": idx})
        elif mode == "wh":
            mm = np.concatenate(
                [mappings[bs].reshape(-1), np.ones(NWP - NW, np.int32)]
            )
            beta = (mm == 2).astype(npdt)
            idx = np.empty((nchunk, 128, cpw // 16), np.int16)
            bt = np.empty((nchunk, 128, jj), npdt)
            for q in range(nchunk):
                sl = slice(q * cpw, (q + 1) * cpw)
                idx[q] = _wrap16(a[sl].reshape(128, jj).T.ravel())
                bt[q] = beta[sl].reshape(128, jj)
            in_maps.append({"x": x, "idx": idx, "bt": bt})
        else:
            m = np.concatenate(
                [mappings[bs].reshape(-1), np.ones(NWP - NW, np.int32)]
            ).astype(np.float32)
            r1 = 1.0 / m
            r2 = (m - 1.0) / m
            rdt = np.float32 if mode == "ws" else npdt
            idx = np.empty((nchunk, 128, cpw // 16), np.int16)
            rw = np.empty((nchunk, 128, 2 * jj), rdt)
            for q in range(nchunk):
                sl = slice(q * cpw, (q + 1) * cpw)
                aq = a[sl].reshape(128, jj).T.ravel()  # i = c*128 + p
                idx[q] = _wrap16(aq)
                rw[q, :, 0::2] = r1[sl].reshape(128, jj).astype(rdt)
                rw[q, :, 1::2] = r2[sl].reshape(128, jj).astype(rdt)
            in_maps.append({"x": x, "idx": idx, "rw": rw})
    return in_maps


def _run(output, mappings, reps=1, bufs=2, order="pc", nq=1, mode="ab",
         nchunk=NCHUNK, merged_idx=False, warm=False, hscale=False,
         sizes=None, dt="f32", split0=False, **kw):
    in_maps = _make_in_maps(output, mappings, order, mode, nchunk, hscale,
                            sizes, dt)
    nc = _get_nc(reps, bufs, order, nq, mode, nchunk, merged_idx, warm,
                 hscale, sizes, dt, split0)
    res = run_bass_kernel_spmd(nc, in_maps, list(range(NCORES)), **kw)
    if mode == "mm":
        outs = [
            np.asarray(r["y"], dtype=np.float32).reshape(BPC, 256, E)[:, :W]
            for r in res.results
        ]
    else:
        outs = [
            np.asarray(r["y"][:NW], dtype=np.float32).reshape(BPC, W, E)
            for r in res.results
        ]
    return np.concatenate(outs, axis=0), res


# Best HW-verified configuration: matmul segment-sum (mode "mm"),
# ~47.3us median (vs 76.3us ab-f32 baseline, 52.7us "wh" bf16 gather).
# Runner-up kept working: dict(bufs=6, order="pc", nq=1, mode="wh",
# nchunk=8, merged_idx=True, split0=True, dt="bf16") @ ~52.7us.
_CFG = dict(mode="mm")


def kernel(output, mappings):
    full, _ = _run(output, mappings, **_CFG)
    return full



# revision 86
# speedup vs baseline: 1.0012x; 1.0012x over previous
"""BERT per-word mean-pool (segment reduce) on 8 Trainium2 NeuronCores.

Problem: output[B=64, S=512, E=768] f32, mappings[B, W=255] int32 (values 1 or 2).
Per sentence, strip [CLS]/[SEP], mean-pool contiguous BPE spans into word vectors.

Key identity: every word's span has 1 or 2 BPE rows.  With s = span start,
    out[w] = (1/m) * (hs rows s .. s+m-1 summed),  m in {1, 2}.

Sharding: pure data parallel, 8 sentences per core, no cross-core comms.
All device data in bf16 (tolerance 2e-2 >> bf16's ~2e-3); host casts.

Two competitive kernels, HW-measured:

* mode "mm" (default): segment-sum as matmul. NO gathers, NO Q7 ucode --
  dodges the ~16us ucode-ready gate that floors every dma_gather kernel.
  Contiguous loads of x (SWDGE+HWDGE alternating); the 0/1 segment matrix
  M^T[t, w] = [s_w <= t] - [s_{w+1} <= t] is built on-chip from an uploaded
  boundary row (2 DVE ops per 128-row k-tile); TensorE accumulates
  out = M^T.T @ x per (sentence, 128-word tile) in PSUM; ACT evicts with a
  per-partition 1/m scale; HWDGE stores.

* mode "wh": windowed ucode gather. One 2-row-window descriptor per word
  (InstDMAGatherAnt), x pre-scaled by 1/m on host, single fused DVE
  scalar_tensor_tensor per word-column: out = t1*[m==2] + t0.

Raw indirect InstDMACopy (modes ind/acc/wind) is broken on this runtime:
the DGE consumes ONE offset per partition of the out AP and fetches
consecutive rows after it (HW-probed); a 3D out AP makes it worse. Do not
use those modes.
"""

import numpy as np

from concourse import bacc, bass, mybir, tile
from concourse.bass_utils import run_bass_kernel_spmd

B, S, W, E = 64, 512, 255, 768
NCORES = 8
BPC = B // NCORES            # sentences per core
NW = BPC * W                 # 2040 real words per core
NWP = 2048                   # padded word count (multiple of 512)
NCHUNK = 4                   # chunks per core
CPW = NWP // NCHUNK          # 512 words per chunk
JJ = CPW // 128              # 4 words per partition per chunk
ROWS = BPC * S               # 4096 input rows per core
NIDX = 2 * CPW               # 1024 gather indices per chunk (A then B)

_F32 = mybir.dt.float32
_BF16 = mybir.dt.bfloat16
_I16 = mybir.dt.int16

_NPBF16 = None


def _np_bf16():
    global _NPBF16
    if _NPBF16 is None:
        import ml_dtypes

        _NPBF16 = ml_dtypes.bfloat16
    return _NPBF16


def _build_nc(reps=1, bufs=2, order="pc", nq=1, mode="ab", nchunk=NCHUNK,
              merged_idx=False, warm=False, hscale=False, sizes=None,
              dt="f32", split0=False):
    _DT = _F32 if dt == "f32" else _BF16
    if sizes is not None:
        return _build_nc_sized(reps, bufs, sizes, hscale)
    if mode == "ind":
        return _build_nc_ind(reps, bufs, dt, nchunk, hscale)
    if mode == "acc":
        return _build_nc_acc(reps, bufs, dt, nchunk)
    if mode == "wind":
        return _build_nc_wind(reps, bufs, dt, nchunk)
    if mode == "mm":
        return _build_nc_mm(reps)
    nc = bacc.Bacc(
        "TRN2",
        target_bir_lowering=False,
        debug=False,
        num_devices=NCORES,
        num_swdge_queues=nq,
    )
    x = nc.dram_tensor("x", [ROWS, E], _DT, kind="ExternalInput").ap()
    # indices are int16, wrapped [i%16, i//16] into 16 partitions and
    # replicated 8x down to 128 partitions (Q7 core replication).
    cpw = NWP // nchunk
    jj = cpw // 128
    nidx = 2 * cpw if mode == "ab" else cpw
    idx = nc.dram_tensor(
        "idx", [nchunk, 128, nidx // 16], _I16, kind="ExternalInput"
    ).ap()
    _RDT = _F32 if mode == "ws" else _DT  # tensor_scalar wants f32 scalars
    if mode in ("win", "ws"):
        # per word w: rw[p, 2c] = 1/m(w), rw[p, 2c+1] = (m(w)-1)/m(w)
        rw = nc.dram_tensor(
            "rw", [nchunk, 128, 2 * jj], _RDT, kind="ExternalInput"
        ).ap()
    if mode == "wh":
        # per word w: bt[p, c] = [m(w) == 2]; x rows pre-scaled by 1/m on host
        bt = nc.dram_tensor(
            "bt", [nchunk, 128, jj], _DT, kind="ExternalInput"
        ).ap()
    y = nc.dram_tensor("y", [NWP, E], _DT, kind="ExternalOutput").ap()

    with tile.TileContext(nc) as tc:
        with (
            tc.tile_pool(name="idxp", bufs=1) as ipool,
            tc.tile_pool(name="io", bufs=bufs) as pool,
        ):
            if warm:
                # dummy 16-index gather issued first: triggers the Q7
                # ucode IRAM fetch (~6us) while the idx loads stream in,
                # so the first real gather isn't stalled on it.
                wi = ipool.tile([128, 1], _I16, tag="warmi")
                nc.gpsimd.memset(wi[:], 0)
                wo = ipool.tile([128, E], _DT, tag="warmo")
                nc.gpsimd.dma_gather(
                    wo[:].rearrange("p (c e) -> p c e", e=E),
                    x[:, :], wi[:], 16, 16, E,
                )
            its, rts = [], []
            ncols = nidx // 16
            if merged_idx:
                its = []
                if split0:
                    # chunk-0 idx as its own tiny first DMA so the first
                    # gather isn't gated on the full idx upload
                    it0 = ipool.tile([128, ncols], _I16, tag="it0")
                    nc.sync.dma_start(out=it0[:], in_=idx[0])
                    its.append(it0[:])
                    itall = ipool.tile(
                        [128, (nchunk - 1) * ncols], _I16, tag="itall"
                    )
                    nc.sync.dma_start(
                        out=itall[:].rearrange(
                            "p (q s) -> p q s", q=nchunk - 1
                        ),
                        in_=idx[1:].rearrange("q p s -> p q s"),
                    )
                    its += [
                        itall[:, q * ncols : (q + 1) * ncols]
                        for q in range(nchunk - 1)
                    ]
                else:
                    itall = ipool.tile([128, nchunk * ncols], _I16, tag="itall")
                    nc.sync.dma_start(
                        out=itall[:].rearrange("p (q s) -> p q s", q=nchunk),
                        in_=idx.rearrange("q p s -> p q s"),
                    )
                    its = [
                        itall[:, q * ncols : (q + 1) * ncols]
                        for q in range(nchunk)
                    ]
            else:
                for q in range(nchunk):
                    it = ipool.tile([128, ncols], _I16, tag=f"it{q}")
                    nc.sync.dma_start(out=it[:], in_=idx[q])
                    its.append(it[:])
            if mode in ("win", "ws"):
                # single merged weight load (one HWDGE DMA for all chunks)
                rtall = ipool.tile([128, nchunk * 2 * jj], _RDT, tag="rtall")
                nc.sync.dma_start(
                    out=rtall[:].rearrange("p (q s) -> p q s", q=nchunk),
                    in_=rw.rearrange("q p s -> p q s"),
                )
                rts = [
                    rtall[:, q * 2 * jj : (q + 1) * 2 * jj]
                    for q in range(nchunk)
                ]
            if mode == "wh":
                btall = ipool.tile([128, nchunk * jj], _DT, tag="btall")
                nc.sync.dma_start(
                    out=btall[:].rearrange("p (q s) -> p q s", q=nchunk),
                    in_=bt.rearrange("q p s -> p q s"),
                )
            for _rep in range(reps):
                for q in range(nchunk):
                    if mode == "ab":
                        # gathered slot i -> T[i % 128, i // 128, :]
                        # i = c*128 + p:  c in 0..3 -> first-BPE row of word
                        # w = q*512 + p*4 + c;  c in 4..7 -> last-BPE row.
                        t = pool.tile([128, 2 * jj * E], _DT, tag="t")
                        nc.gpsimd.dma_gather(
                            t[:].rearrange("p (c e) -> p c e", e=E),
                            x[:, :],
                            its[q],
                            nidx,
                            nidx,
                            E,
                            queue_num=q % nq,
                        )
                        c = pool.tile([128, jj * E], _DT, tag="c")
                        nc.vector.tensor_add(
                            out=c[:], in0=t[:, : jj * E], in1=t[:, jj * E :]
                        )
                        if not hscale:
                            nc.scalar.mul(c[:], c[:], 0.5)
                    else:
                        # one 2-row window [s, s+2) per word, 6KB descriptors;
                        # out[w] = win[0]*r1 + win[1]*r2 kills the junk row
                        # (m=1: r=(1,0); m=2: r=(.5,.5)).
                        t = pool.tile([128, 2 * jj * E], _DT, tag="t")
                        xw = bass.AP(x.tensor, 0, [[E, ROWS - 1], [1, 2 * E]])
                        nc.gpsimd.dma_gather(
                            t[:].rearrange("p (c e) -> p c e", e=2 * E),
                            xw,
                            its[q],
                            cpw,
                            cpw,
                            2 * E,
                            elem_step=E,
                            queue_num=q % nq,
                        )
                        c = pool.tile([128, jj * E], _DT, tag="c")
                        if mode == "wh":
                            # x rows pre-scaled by 1/m on host; one fused DVE
                            # op per word-column: out = t1*[m==2] + t0, all
                            # APs contiguous [128, E] (full-rate DVE).
                            for j in range(jj):
                                nc.vector.scalar_tensor_tensor(
                                    out=c[:, j * E : (j + 1) * E],
                                    in0=t[:, (2 * j + 1) * E : (2 * j + 2) * E],
                                    scalar=btall[:, q * jj + j : q * jj + j + 1],
                                    in1=t[:, 2 * j * E : (2 * j + 1) * E],
                                    op0=mybir.AluOpType.mult,
                                    op1=mybir.AluOpType.add,
                                )
                        elif mode == "ws":
                            # device weights, contiguous slices: per word-col
                            # c3 = t1*r2; c = t0*r1 + c3
                            c3 = pool.tile([128, jj * E], _DT, tag="c3")
                            for j in range(jj):
                                nc.vector.tensor_scalar_mul(
                                    c3[:, j * E : (j + 1) * E],
                                    t[:, (2 * j + 1) * E : (2 * j + 2) * E],
                                    rts[q][:, 2 * j + 1 : 2 * j + 2],
                                )
                                nc.vector.scalar_tensor_tensor(
                                    out=c[:, j * E : (j + 1) * E],
                                    in0=t[:, 2 * j * E : (2 * j + 1) * E],
                                    scalar=rts[q][:, 2 * j : 2 * j + 1],
                                    in1=c3[:, j * E : (j + 1) * E],
                                    op0=mybir.AluOpType.mult,
                                    op1=mybir.AluOpType.add,
                                )
                        else:
                            t3 = t[:].rearrange("p (c e) -> p c e", e=2 * E)
                            r3 = rts[q].rearrange("p (c f) -> p c f", f=2)
                            c3 = pool.tile([128, jj * E], _DT, tag="c3")
                            cv = c[:].rearrange("p (j e) -> p j e", e=E)
                            c3v = c3[:].rearrange("p (j e) -> p j e", e=E)
                            nc.vector.tensor_tensor(
                                out=cv,
                                in0=t3[:, :, :E],
                                in1=r3[:, :, 0:1].to_broadcast([128, jj, E]),
                                op=mybir.AluOpType.mult,
                            )
                            nc.vector.tensor_tensor(
                                out=c3v,
                                in0=t3[:, :, E:],
                                in1=r3[:, :, 1:2].to_broadcast([128, jj, E]),
                                op=mybir.AluOpType.mult,
                            )
                            nc.vector.tensor_add(out=c[:], in0=c[:], in1=c3[:])
                    ychunk = y[q * cpw : (q + 1) * cpw, :]
                    if order == "pc":
                        nc.sync.dma_start(
                            out=ychunk.rearrange("(p j) e -> p (j e)", p=128),
                            in_=c[:],
                        )
                    else:
                        nc.sync.dma_start(
                            out=ychunk.rearrange("(j p) e -> p j e", p=128),
                            in_=c[:].rearrange("p (j e) -> p j e", e=E),
                        )
    nc.compile()
    return nc


def _build_nc_ind(reps, bufs, dt="f32", nchunk=4, hscale=False, tbufs=1):
    """Native SWDGE indirect gather (no ucode library -> no ~16us Q7 ucode
    ready gate). One InstDMACopy per chunk reading a column slice of a single
    int32 offset tile; slices advance monotonically (the HW-verified-safe
    pattern). Per chunk: first wpp cols = first-BPE rows, next wpp cols =
    last-BPE rows of words w = q*cpw + p*wpp + j."""
    _DT = _F32 if dt == "f32" else _BF16
    cpw = NWP // nchunk
    wpp = cpw // 128              # words per partition per chunk
    nc = bacc.Bacc(
        "TRN2", target_bir_lowering=False, debug=False, num_devices=NCORES
    )
    x = nc.dram_tensor("x", [ROWS, E], _DT, kind="ExternalInput").ap()
    idx = nc.dram_tensor("idx", [128, 2 * wpp * nchunk], mybir.dt.int32,
                         kind="ExternalInput").ap()
    y = nc.dram_tensor("y", [NWP, E], _DT, kind="ExternalOutput").ap()
    with tile.TileContext(nc) as tc:
        with (
            tc.tile_pool(name="idxp", bufs=1) as ipool,
            tc.tile_pool(name="tp", bufs=tbufs) as tpool,
            tc.tile_pool(name="io", bufs=bufs) as pool,
        ):
            itall = ipool.tile([128, 2 * wpp * nchunk], mybir.dt.int32,
                               tag="itall")
            nc.sync.dma_start(out=itall[:], in_=idx[:, :])
            for _rep in range(reps):
                for q in range(nchunk):
                    # tbufs=1 pool: WAR dep guarantees at most one indirect
                    # DMA in flight (two concurrent ones corrupt offsets).
                    t = tpool.tile([128, 2 * wpp * E], _DT, tag="t")
                    nc.gpsimd.indirect_dma_start(
                        # 3D out AP: one offset consumed per (p, c) row slot.
                        # A flat [128, 2*wpp*E] out makes the DGE take ONE
                        # offset per partition and fetch consecutive rows
                        # (HW-probed failure mode).
                        out=t[:].rearrange("p (c e) -> p c e", e=E),
                        out_offset=None,
                        in_=x[:, :],
                        in_offset=bass.IndirectOffsetOnAxis(
                            ap=itall[:, 2 * wpp * q : 2 * wpp * (q + 1)],
                            axis=0,
                        ),
                    )
                    c = pool.tile([128, wpp * E], _DT, tag="c")
                    nc.vector.tensor_add(
                        out=c[:], in0=t[:, : wpp * E], in1=t[:, wpp * E :]
                    )
                    if not hscale:
                        nc.scalar.mul(c[:], c[:], 0.5)
                    nc.sync.dma_start(
                        out=y[q * cpw : (q + 1) * cpw, :].rearrange(
                            "(p j) e -> p (j e)", p=128
                        ),
                        in_=c[:],
                    )
    nc.compile()
    return nc


def _build_nc_wind(reps, bufs, dt, nchunk, tbufs=1):
    """Native indirect gather of 2-row windows (one InstDMACopy per chunk,
    offsets in raw elements via a 1-D source view; coef=1), then the wh-style
    fused STT combine (x host-scaled by 1/m, beta kills junk rows)."""
    _DT = _F32 if dt == "f32" else _BF16
    cpw = NWP // nchunk
    wpp = cpw // 128
    nc = bacc.Bacc(
        "TRN2", target_bir_lowering=False, debug=False, num_devices=NCORES
    )
    x = nc.dram_tensor("x", [ROWS, E], _DT, kind="ExternalInput").ap()
    idx = nc.dram_tensor("idx", [128, wpp * nchunk], mybir.dt.int32,
                         kind="ExternalInput").ap()
    bt = nc.dram_tensor("bt", [nchunk, 128, wpp], _DT,
                        kind="ExternalInput").ap()
    y = nc.dram_tensor("y", [NWP, E], _DT, kind="ExternalOutput").ap()
    with tile.TileContext(nc) as tc:
        with (
            tc.tile_pool(name="idxp", bufs=1) as ipool,
            tc.tile_pool(name="tp", bufs=tbufs) as tpool,
            tc.tile_pool(name="io", bufs=bufs) as pool,
        ):
            itall = ipool.tile([128, wpp * nchunk], mybir.dt.int32,
                               tag="itall")
            nc.sync.dma_start(out=itall[:], in_=idx[:, :])
            btall = ipool.tile([128, nchunk * wpp], _DT, tag="btall")
            nc.sync.dma_start(
                out=btall[:].rearrange("p (q s) -> p q s", q=nchunk),
                in_=bt.rearrange("q p s -> p q s"),
            )
            for _rep in range(reps):
                for q in range(nchunk):
                    t = tpool.tile([128, 2 * wpp * E], _DT, tag="t")
                    # coef = E (row stride); each (p, c) out slot pulls 2E
                    # contiguous elements = the 2-row window [s, s+2)
                    nc.gpsimd.indirect_dma_start(
                        out=t[:].rearrange("p (c e) -> p c e", e=2 * E),
                        out_offset=None,
                        in_=x[:, :],
                        in_offset=bass.IndirectOffsetOnAxis(
                            ap=itall[:, wpp * q : wpp * (q + 1)], axis=0),
                    )
                    c = pool.tile([128, wpp * E], _DT, tag="c")
                    for j in range(wpp):
                        nc.vector.scalar_tensor_tensor(
                            out=c[:, j * E : (j + 1) * E],
                            in0=t[:, (2 * j + 1) * E : (2 * j + 2) * E],
                            scalar=btall[:, q * wpp + j : q * wpp + j + 1],
                            in1=t[:, 2 * j * E : (2 * j + 1) * E],
                            op0=mybir.AluOpType.mult,
                            op1=mybir.AluOpType.add,
                        )
                    nc.sync.dma_start(
                        out=y[q * cpw : (q + 1) * cpw, :].rearrange(
                            "(p j) e -> p (j e)", p=128),
                        in_=c[:],
                    )
    nc.compile()
    return nc


def _build_nc_mm(reps=1, xbufs=1, pbufs=4, obufs=6):
    """Segment-sum as matmul: NO gathers, NO Q7 ucode (dodges the ~16us
    ucode-ready gate). Contiguous HWDGE loads of x; the 0/1 segment matrix
    M^T[t, w] = [s_w <= t] - [s_{w+1} <= t] is built on-chip from an
    uploaded (replicated) boundary row via two DVE ops per k-tile; TensorE
    computes out = M^T.T @ x per (sentence, word-tile); PSUM is evicted with
    a per-partition 1/m scale (the span mean) straight to bf16 and stored.

    Word layout: y row = sent*256 + w (pad word 255 per sentence interleaved).
    """
    NS = BPC                      # sentences per core = 8
    KT = S // 128                 # k-tiles per sentence = 4
    WT = 2                        # word-tiles per sentence (256 words)
    SW = 257                      # boundary cols per sentence (s_0..s_255, sentinel)
    nc = bacc.Bacc(
        "TRN2", target_bir_lowering=False, debug=False, num_devices=NCORES
    )
    x = nc.dram_tensor("x", [ROWS, E], _BF16, kind="ExternalInput").ap()
    sth = nc.dram_tensor("sth", [1, NS * SW], mybir.dt.float16,
                         kind="ExternalInput").ap()
    gcol = nc.dram_tensor("gcol", [128, KT], _F32, kind="ExternalInput").ap()
    msc = nc.dram_tensor("msc", [128, NS * WT], _F32,
                         kind="ExternalInput").ap()
    y = nc.dram_tensor("y", [NS * 256, E], _BF16, kind="ExternalOutput").ap()

    with tile.TileContext(nc) as tc:
        with (
            tc.tile_pool(name="const", bufs=1) as cpool,
            tc.tile_pool(name="xs", bufs=xbufs) as xpool,
            tc.tile_pool(name="m", bufs=1) as mpool,
            tc.tile_pool(name="ps", bufs=pbufs, space="PSUM") as ppool,
            tc.tile_pool(name="out", bufs=obufs) as opool,
        ):
            # x loads split across the SWDGE ring (gpsimd) and the ACT
            # engine's own HWDGE ring (qActDynamicHW) -- two descriptor
            # paths inject in parallel; the sync ring stays FIFO-clean for
            # the stores (mixing big loads with stores on one ring
            # head-of-line-blocks them; HW-measured +14us)
            # boundary row: 4KB flat upload + PE broadcast to 128 partitions
            # (a [128, 2056] replicated upload costs 0.53MB of HBM stream;
            # ones-matmul replication costs ~0)
            sfl = cpool.tile([1, NS * SW], mybir.dt.float16, tag="sfl")
            nc.sync.dma_start(out=sfl[:], in_=sth[:, :])
            ones = cpool.tile([1, 128], mybir.dt.float16, tag="ones")
            nc.vector.memset(ones[:], 1.0)
            stht = cpool.tile([128, NS * SW], mybir.dt.float16, tag="sth")
            off = 0
            while off < NS * SW:
                n = min(384, NS * SW - off)
                pb = ppool.tile([128, 384], _F32, tag="p0")
                nc.tensor.matmul(
                    pb[:, :n], ones[:], sfl[:, off : off + n],
                    start=True, stop=True,
                )
                nc.vector.tensor_copy(out=stht[:, off : off + n], in_=pb[:, :n])
                off += n
            gct = cpool.tile([128, KT], _F32, tag="gcol")
            nc.sync.dma_start(out=gct[:], in_=gcol[:, :])
            msct = cpool.tile([128, NS * WT], _F32, tag="msc")
            nc.sync.dma_start(out=msct[:], in_=msc[:, :])
            # last sentence per ring (s6 scalar, s7 gpsimd) split (k0-2)+(k3):
            # its wt0 matmuls overlap the final k3 transfer, shortening the
            # post-last-load chain
            xts = []
            xbs = {}
            for s in range(NS):
                eng = nc.scalar if s % 2 == 0 else nc.gpsimd
                xin = x[s * S : (s + 1) * S, :].rearrange(
                    "(k p) e -> p k e", p=128
                )
                if s >= NS - 2:
                    xa = xpool.tile([128, 3 * E], _BF16, tag=f"xa{s}")
                    eng.dma_start(
                        out=xa[:].rearrange("p (k e) -> p k e", e=E),
                        in_=x[s * S : s * S + 384, :].rearrange(
                            "(k p) e -> p k e", p=128
                        ),
                    )
                    xb = xpool.tile([128, E], _BF16, tag=f"xb{s}")
                    eng.dma_start(out=xb[:], in_=x[s * S + 384 : (s + 1) * S, :])
                    xts.append(xa)
                    xbs[s] = xb
                else:
                    xt = xpool.tile([128, KT * E], _BF16, tag=f"x{s}")
                    eng.dma_start(
                        out=xt[:].rearrange("p (k e) -> p k e", e=E),
                        in_=xin,
                    )
                    xts.append(xt)
            for _rep in range(reps):
                # cmp_k[p, sent*SW + w] = [s_w <= p + 128k]  (0/1 bf16)
                cmps = []
                for k in range(KT):
                    ck = mpool.tile([128, NS * SW], _BF16, tag=f"cmp{k}")
                    nc.vector.tensor_scalar(
                        ck[:], stht[:], gct[:, k : k + 1], None,
                        op0=mybir.AluOpType.is_le,
                    )
                    cmps.append(ck)
                # M^T block (k, sent): [128 rows, 256 words]
                mts = {}
                for k in range(KT):
                    for s in range(NS):
                        mt = mpool.tile([128, 256], _BF16, tag=f"m{k}_{s}")
                        nc.vector.tensor_tensor(
                            out=mt[:],
                            in0=cmps[k][:, s * SW : s * SW + 256],
                            in1=cmps[k][:, s * SW + 1 : s * SW + 257],
                            op=mybir.AluOpType.subtract,
                        )
                        mts[(k, s)] = mt
                # per (sent, wt): 3 k-blocks x 2 N-halves accumulate in PSUM
                for s in range(NS):
                    for wt in range(WT):
                        ks = (0, 1, 2) if wt == 0 else (1, 2, 3)
                        pts = []
                        for h in range(2):
                            pt = ppool.tile([128, 384], _F32, tag=f"p{h}")
                            for i, k in enumerate(ks):
                                if k == 3 and s in xbs:
                                    rhs = xbs[s][:, h * 384 : (h + 1) * 384]
                                else:
                                    rhs = xts[s][:, k * E + h * 384 :
                                                 k * E + (h + 1) * 384]
                                nc.tensor.matmul(
                                    pt[:],
                                    mts[(k, s)][:, wt * 128 : (wt + 1) * 128],
                                    rhs,
                                    start=(i == 0),
                                    stop=(i == len(ks) - 1),
                                )
                            pts.append(pt)
                        ot = opool.tile([128, E], _BF16, tag="o")
                        col = s * WT + wt
                        # PSUM eviction + 1/m scale split across DVE and ACT
                        # (a single engine's 32-op eviction chain paces the
                        # whole tail at ~20us; HW-traced)
                        nc.vector.tensor_scalar_mul(
                            ot[:, 0:384], pts[0][:], msct[:, col : col + 1]
                        )
                        nc.scalar.mul(
                            ot[:, 384:768], pts[1][:], msct[:, col : col + 1]
                        )
                        nc.sync.dma_start(
                            out=y[s * 256 + wt * 128 : s * 256 + (wt + 1) * 128, :],
                            in_=ot[:],
                        )
    nc.compile()
    return nc


def _build_nc_acc(reps, bufs, dt="f32", nchunk=4):
    """Native indirect gathers, zero compute engines: host uploads x/2, the
    B gather CCE-accumulates onto the A gather in SBUF, and the only
    consumer is a DMA store (the HW-verified-safe consumer)."""
    _DT = _F32 if dt == "f32" else _BF16
    cpw = NWP // nchunk
    wpp = cpw // 128
    nc = bacc.Bacc(
        "TRN2", target_bir_lowering=False, debug=False, num_devices=NCORES
    )
    x = nc.dram_tensor("x", [ROWS, E], _DT, kind="ExternalInput").ap()
    idx = nc.dram_tensor("idx", [128, 2 * wpp * nchunk], mybir.dt.int32,
                         kind="ExternalInput").ap()
    y = nc.dram_tensor("y", [NWP, E], _DT, kind="ExternalOutput").ap()
    with tile.TileContext(nc) as tc:
        with (
            tc.tile_pool(name="idxp", bufs=1) as ipool,
            tc.tile_pool(name="io", bufs=bufs) as pool,
        ):
            itall = ipool.tile([128, 2 * wpp * nchunk], mybir.dt.int32,
                               tag="itall")
            nc.sync.dma_start(out=itall[:], in_=idx[:, :])
            for _rep in range(reps):
                for q in range(nchunk):
                    o = 2 * wpp * q
                    t = pool.tile([128, wpp * E], _DT, tag="t")
                    t3 = t[:].rearrange("p (c e) -> p c e", e=E)
                    nc.gpsimd.indirect_dma_start(
                        out=t3, out_offset=None, in_=x[:, :],
                        in_offset=bass.IndirectOffsetOnAxis(
                            ap=itall[:, o : o + wpp], axis=0),
                    )
                    nc.gpsimd.indirect_dma_start(
                        out=t3, out_offset=None, in_=x[:, :],
                        in_offset=bass.IndirectOffsetOnAxis(
                            ap=itall[:, o + wpp : o + 2 * wpp], axis=0),
                        compute_op=mybir.AluOpType.add,
                    )
                    nc.sync.dma_start(
                        out=y[q * cpw : (q + 1) * cpw, :].rearrange(
                            "(p j) e -> p (j e)", p=128),
                        in_=t[:],
                    )
    nc.compile()
    return nc


def _build_nc_sized(reps, bufs, sizes, hscale):
    """ab-mode kernel with per-chunk word counts `sizes` (multiples of 128
    summing to NWP). Small leading chunks start transfers sooner; small
    trailing chunks shorten the exposed compute+store tail."""
    assert sum(sizes) == NWP and all(s % 128 == 0 for s in sizes)
    nc = bacc.Bacc(
        "TRN2", target_bir_lowering=False, debug=False, num_devices=NCORES
    )
    x = nc.dram_tensor("x", [ROWS, E], _F32, kind="ExternalInput").ap()
    total_cols = sum(2 * s // 16 for s in sizes)
    idx = nc.dram_tensor("idx", [128, total_cols], _I16, kind="ExternalInput").ap()
    y = nc.dram_tensor("y", [NWP, E], _F32, kind="ExternalOutput").ap()
    jmax = max(sizes) // 128

    with tile.TileContext(nc) as tc:
        with (
            tc.tile_pool(name="idxp", bufs=1) as ipool,
            tc.tile_pool(name="io", bufs=bufs) as pool,
        ):
            itall = ipool.tile([128, total_cols], _I16, tag="itall")
            nc.sync.dma_start(out=itall[:], in_=idx[:, :])
            for _rep in range(reps):
                off_c = 0
                off_w = 0
                for s in sizes:
                    jj = s // 128
                    cols = 2 * s // 16
                    t = pool.tile([128, 2 * jmax * E], _F32, tag="t")
                    nc.gpsimd.dma_gather(
                        t[:, : 2 * jj * E].rearrange("p (c e) -> p c e", e=E),
                        x[:, :],
                        itall[:, off_c : off_c + cols],
                        2 * s,
                        2 * s,
                        E,
                    )
                    c = pool.tile([128, jmax * E], _F32, tag="c")
                    nc.vector.tensor_add(
                        out=c[:, : jj * E],
                        in0=t[:, : jj * E],
                        in1=t[:, jj * E : 2 * jj * E],
                    )
                    if not hscale:
                        nc.scalar.mul(c[:, : jj * E], c[:, : jj * E], 0.5)
                    nc.sync.dma_start(
                        out=y[off_w : off_w + s, :].rearrange(
                            "(p j) e -> p (j e)", p=128
                        ),
                        in_=c[:, : jj * E],
                    )
                    off_c += cols
                    off_w += s
    nc.compile()
    return nc


_NC = {}


def _get_nc(reps=1, bufs=2, order="pc", nq=1, mode="ab", nchunk=NCHUNK,
            merged_idx=False, warm=False, hscale=False, sizes=None,
            dt="f32", split0=False):
    key = (reps, bufs, order, nq, mode, nchunk, merged_idx, warm, hscale,
           tuple(sizes) if sizes else None, dt, split0)
    if key not in _NC:
        _NC[key] = _build_nc(reps, bufs, order, nq, mode, nchunk, merged_idx,
                             warm, hscale, sizes, dt, split0)
    return _NC[key]


def _wrap16(flat):
    """int16 index list -> [128, n/16] wrapped (i -> [i%16, i//16]) + 8x rep."""
    return np.tile(flat.reshape(-1, 16).T, (8, 1)).astype(np.int16)


def _make_in_maps(output, mappings, order="pc", mode="ab", nchunk=NCHUNK,
                  hscale=False, sizes=None, dt="f32"):
    output = np.asarray(output)
    if hscale:
        # fold the *0.5 of the span mean into the shard upload: a/2 + b/2
        # rounds identically to (a+b)/2 in f32 (halving is exact).
        output = output * np.float32(0.5)
    npdt = np.float32 if dt == "f32" else _np_bf16()
    mappings = np.asarray(mappings, dtype=np.int32)
    ends = np.cumsum(mappings, axis=1, dtype=np.int32)  # [B, W] exclusive ends
    src_a = ends - mappings + 1                         # +1: skip [CLS]
    src_b = ends                                        # (e-1) + 1
    if mode in ("wh", "wind"):
        # scale every BPE row by 1/m of its owning word (0.5/1 exact in f32,
        # single bf16 rounding after); junk rows keep scale 1 and are killed
        # on-device by beta=0.
        g = np.ones((B, S), np.float32)
        two = mappings == 2
        np.put_along_axis(
            g, src_a, np.where(two, np.float32(0.5), np.float32(1.0)), axis=1
        )
        bi, wi_ = np.nonzero(two)
        g[bi, src_a[bi, wi_] + 1] = 0.5
        output = output * g[:, :, None]
    output = np.ascontiguousarray(output.astype(npdt))

    if mode == "mm":
        in_maps = []
        gcol = (np.arange(128, dtype=np.float32)[:, None]
                + 128.0 * np.arange(4, dtype=np.float32)[None, :])
        gcol = np.ascontiguousarray(gcol)
        minv = 1.0 / mappings.astype(np.float32)          # [B, W]
        for k in range(NCORES):
            bs = slice(k * BPC, (k + 1) * BPC)
            sa = src_a[bs]                                # [8, 255] s-coords
            sth = np.empty((BPC, 257), np.float16)
            sth[:, :255] = sa
            sth[:, 255] = ends[bs, -1] + 1                # pad word start
            sth[:, 256] = 600.0                           # sentinel > 511
            sth = np.ascontiguousarray(sth.reshape(1, -1))  # [1, 2056]
            msc = np.ones((128, BPC * 2), np.float32)
            mi = minv[bs]                                 # [8, 255]
            for s_ in range(BPC):
                msc[:, s_ * 2] = mi[s_, 0:128]
                msc[:128 - 1, s_ * 2 + 1] = mi[s_, 128:255]
            xk = np.ascontiguousarray(
                output[bs].reshape(ROWS, E).astype(_np_bf16()))
            in_maps.append({"x": xk, "sth": sth, "gcol": gcol,
                            "msc": np.ascontiguousarray(msc)})
        return in_maps

    in_maps = []
    for k in range(NCORES):
        bs = slice(k * BPC, (k + 1) * BPC)
        base = (np.arange(BPC, dtype=np.int32) * S)[:, None]
        a = (src_a[bs] + base).reshape(-1)
        b = (src_b[bs] + base).reshape(-1)
        pad = np.zeros(NWP - NW, np.int32)
        a = np.concatenate([a, pad])  # [NWP] word-ordered flat row ids
        b = np.concatenate([b, pad])
        x = np.ascontiguousarray(output[bs].reshape(ROWS, E))
        if mode == "wind":
            wpp = NWP // nchunk // 128
            mm = np.concatenate(
                [mappings[bs].reshape(-1), np.ones(NWP - NW, np.int32)]
            )
            beta = (mm == 2).astype(npdt)
            # [p, q*wpp+j] = element offset of word q*cpw + p*wpp + j
            aw = a.reshape(nchunk, 128, wpp).transpose(1, 0, 2)  # [p, q, j]
            idx = np.ascontiguousarray(aw.reshape(128, -1).astype(np.int32))
            bt = np.empty((nchunk, 128, wpp), npdt)
            cpw_ = NWP // nchunk
            for q in range(nchunk):
                sl = slice(q * cpw_, (q + 1) * cpw_)
                bt[q] = beta[sl].reshape(128, wpp)
            in_maps.append({"x": x, "idx": idx, "bt": bt})
            continue
        if mode in ("ind", "acc"):
            nck = nchunk
            wpp = NWP // nck // 128
            ia = a.reshape(nck, 128, wpp).transpose(1, 0, 2)  # [p, q, j]
            ib = b.reshape(nck, 128, wpp).transpose(1, 0, 2)
            idx = np.concatenate(
                [np.concatenate([ia[:, q], ib[:, q]], axis=1)
                 for q in range(nck)],
                axis=1,
            ).astype(np.int32)  # [128, 2*wpp*nck], cols 2*wpp*q + j
            in_maps.append({"x": x, "idx": np.ascontiguousarray(idx)})
            continue
        if sizes is not None:
            segs = []
            off = 0
            for s in sizes:
                jj = s // 128
                aq = a[off : off + s].reshape(128, jj).T.ravel()
                bq = b[off : off + s].reshape(128, jj).T.ravel()
                segs.append(_wrap16(np.concatenate([aq, bq])))
                off += s
            in_maps.append({"x": x, "idx": np.concatenate(segs, axis=1)})
            continue
        cpw = NWP // nchunk
        jj = cpw // 128
        if mode == "ab":
            idx = np.empty((nchunk, 128, 2 * cpw // 16), np.int16)
            for q in range(nchunk):
                aq = a[q * cpw : (q + 1) * cpw]
                bq = b[q * cpw : (q + 1) * cpw]
                if order == "pc":
                    # gathered i = c*128 + p holds word q*cpw + p*jj + c
                    aq = aq.reshape(128, jj).T.ravel()
                    bq = bq.reshape(128, jj).T.ravel()
                # 'seq': gathered i holds word q*cpw + i (ascending rows)
                idx[q] = _wrap16(np.concatenate([aq, bq]))
            in_maps.append({"x": x, "idx": idx})
        elif mode == "wh":
            mm = np.concatenate(
                [mappings[bs].reshape(-1), np.ones(NWP - NW, np.int32)]
            )
            beta = (mm == 2).astype(npdt)
            idx = np.empty((nchunk, 128, cpw // 16), np.int16)
            bt = np.empty((nchunk, 128, jj), npdt)
            for q in range(nchunk):
                sl = slice(q * cpw, (q + 1) * cpw)
                idx[q] = _wrap16(a[sl].reshape(128, jj).T.ravel())
                bt[q] = beta[sl].reshape(128, jj)
            in_maps.append({"x": x, "idx": idx, "bt": bt})
        else:
            m = np.concatenate(
                [mappings[bs].reshape(-1), np.ones(NWP - NW, np.int32)]
            ).astype(np.float32)
            r1 = 1.0 / m
            r2 = (m - 1.0) / m
            rdt = np.float32 if mode == "ws" else npdt
            idx = np.empty((nchunk, 128, cpw // 16), np.int16)
            rw = np.empty((nchunk, 128, 2 * jj), rdt)
            for q in range(nchunk):
                sl = slice(q * cpw, (q + 1) * cpw)
                aq = a[sl].reshape(128, jj).T.ravel()  # i = c*128 + p
                idx[q] = _wrap16(aq)
                rw[q, :, 0::2] = r1[sl].reshape(128, jj).astype(rdt)
                rw[q, :, 1::2] = r2[sl].reshape(128, jj).astype(rdt)
            in_maps.append({"x": x, "idx": idx, "rw": rw})
    return in_maps


def _run(output, mappings, reps=1, bufs=2, order="pc", nq=1, mode="ab",
         nchunk=NCHUNK, merged_idx=False, warm=False, hscale=False,
         sizes=None, dt="f32", split0=False, **kw):
    in_maps = _make_in_maps(output, mappings, order, mode, nchunk, hscale,
                            sizes, dt)
    nc = _get_nc(reps, bufs, order, nq, mode, nchunk, merged_idx, warm,
                 hscale, sizes, dt, split0)
    res = run_bass_kernel_spmd(nc, in_maps, list(range(NCORES)), **kw)
    if mode == "mm":
        outs = [
            np.asarray(r["y"], dtype=np.float32).reshape(BPC, 256, E)[:, :W]
            for r in res.results
        ]
    else:
        outs = [
            np.asarray(r["y"][:NW], dtype=np.float32).reshape(BPC, W, E)
            for r in res.results
        ]
    return np.concatenate(outs, axis=0), res


# Best HW-verified configuration: matmul segment-sum (mode "mm"),
# ~47.3us median (vs 76.3us ab-f32 baseline, 52.7us "wh" bf16 gather).
# Runner-up kept working: dict(bufs=6, order="pc", nq=1, mode="wh",
# nchunk=8, merged_idx=True, split0=True, dt="bf16") @ ~52.7us.
_CFG = dict(mode="mm")


def kernel(output, mappings):
    full, _ = _run(output, mappings, **_CFG)
    return full

